# revision 13
# baseline (speedup 1.0000x reference)
"""Trainium2 Bass kernel for the ALM-terms loss function (v3).

Reference (P=4096 pos, N=8192 neg, delta=0.1):
    q[i]        = sum_j relu(neg[j] - pos[i] + delta)
    new_lambdas = lambdas.at[idx].add(mu * q)          # idx is arange
    loss        = (mu/2 * q[-1]^2 + lambdas[idx[-1]] * q[-1]) / (P*N)

Sharding: pos split 8-way (512 rows/core); neg replicated. No collectives.

Per-core algorithm (all orientation B, fp16 margins):
  - tneg[p, i] = delta - pos[i]  replicated across partitions ([128,512] fp16)
  - neg as 64 columns of 128 ([128, k]); VectorE computes
    m = max(tneg + neg_k, 0) (fp16, 4x mode) for K_V columns; ScalarE
    computes m = Relu(tneg + neg_c) via activation for the rest.
  - TensorE contracts each m column over partitions with a ones-vector
    (fp16 rhs, fp32 PSUM) accumulating all 64 columns into one
    PSUM row [1, 512] = q for this shard.
  - Tail: q -> SBUF, newlam = q*mu + lam in row layout, DMA out.
"""

import numpy as np

import concourse.bass as bass
import concourse.mybir as mybir
from concourse.bass_utils import run_bass_kernel_spmd

P_TOTAL = 4096
N_NEG = 8192
N_CORES = 8
P_SHARD = P_TOTAL // N_CORES       # 512
DELTA = 0.1

N_COLS = N_NEG // 128              # 64 columns of 128 negs
K_V = 47                           # vector columns
K_S = N_COLS - K_V                 # scalar columns

F32 = mybir.dt.float32
F16 = mybir.dt.float16


def _interleave_schedule():
    """TensorE consumption order: list of ('v'|'s', col_idx), Bresenham mix."""
    sched = []
    iv = isch = 0
    for _ in range(N_COLS):
        if isch * K_V >= iv * K_S - 1e-9 and iv < K_V:
            sched.append(("v", iv))
            iv += 1
        elif isch < K_S:
            sched.append(("s", isch))
            isch += 1
        else:
            sched.append(("v", iv))
            iv += 1
    return sched


def build_kernel():
    nc = bass.Bass()

    pos_ext = nc.declare_dram_parameter("pos_row", [1, P_SHARD], F32, False)
    negv_ext = nc.declare_dram_parameter("negv", [128, K_V], F32, False)
    negs_ext = nc.declare_dram_parameter("negs", [128, K_S], F32, False)
    lam_ext = nc.declare_dram_parameter("lam_row", [1, P_SHARD], F32, False)
    mu_ext = nc.declare_dram_parameter("mu", [1, 1], F32, False)
    q_ext = nc.declare_dram_parameter("q_row", [1, P_SHARD], F32, True)
    newlam_ext = nc.declare_dram_parameter("newlam_row", [1, P_SHARD], F32, True)

    sched = _interleave_schedule()
    pos_of_v = {k: i for i, (t, k) in enumerate(sched) if t == "v"}
    pos_of_s = {c: i for i, (t, c) in enumerate(sched) if t == "s"}

    from contextlib import ExitStack

    with ExitStack() as ctx:
        sb = lambda name, shape, dt=F32: ctx.enter_context(
            nc.sbuf_tensor(name, shape, dt)
        )
        pos_rep = sb("pos_rep", [128, P_SHARD])
        tneg = sb("tneg", [128, P_SHARD], F16)
        negv_sb = sb("negv_sb", [128, K_V])
        negs_sb = sb("negs_sb", [128, K_S])
        mv = [sb("mv0", [128, P_SHARD], F16), sb("mv1", [128, P_SHARD], F16)]
        ms = [sb("ms0", [128, P_SHARD], F16), sb("ms1", [128, P_SHARD], F16)]
        ones_t = sb("ones_t", [128, 1], F16)
        q_sb = sb("q_sb", [1, P_SHARD])
        lam_sb = sb("lam_sb", [1, P_SHARD])
        newlam_sb = sb("newlam_sb", [1, P_SHARD])
        mu_sb = sb("mu_sb", [1, 1])
        dum_sb = sb("dum_sb", [1, 1])
        psum_q = ctx.enter_context(nc.psum_tensor("psum_q", [1, P_SHARD], F32))
        sem = lambda name: ctx.enter_context(nc.semaphore(name))
        p_sem = sem("p_sem")
        nv_sem = sem("nv_sem")
        ns_sem = sem("ns_sem")
        misc_sem = sem("misc_sem")
        v_sem = sem("v_sem")
        s_sem = sem("s_sem")
        t_sem = sem("t_sem")
        done_sem = sem("done_sem")
        out_sem = sem("out_sem")
        block = ctx.enter_context(nc.Block())

        @block.sync
        def _(sync):
            sync.dma_start(
                out=pos_rep[:, :], in_=pos_ext[0:1, :].broadcast_to((128, P_SHARD))
            ).then_inc(p_sem, 16)
            sync.dma_start(out=negv_sb[:, :], in_=negv_ext[:, :]).then_inc(nv_sem, 16)
            sync.dma_start(out=negs_sb[:, :], in_=negs_ext[:, :]).then_inc(ns_sem, 16)
            sync.dma_start(out=lam_sb[0:1, :], in_=lam_ext[0:1, :]).then_inc(
                misc_sem, 16
            )
            sync.dma_start(out=mu_sb[0:1, 0:1], in_=mu_ext[0:1, 0:1]).then_inc(
                misc_sem, 16
            )
            sync.wait_ge(done_sem, 1)
            sync.dma_start(out=q_ext[0:1, :], in_=q_sb[0:1, :]).then_inc(out_sem, 16)
            sync.dma_start(out=newlam_ext[0:1, :], in_=newlam_sb[0:1, :]).then_inc(
                out_sem, 16
            )

        @block.scalar
        def _(scalar):
            # dummy ACT pulls the Relu table load off the critical path
            scalar.wait_ge(p_sem, 16)
            scalar.activation(
                dum_sb[0:1, 0:1],
                pos_rep[0:1, 0:1],
                mybir.ActivationFunctionType.Relu,
                bias=0.0,
                scale=1.0,
            )
            scalar.wait_ge(v_sem, 2)  # tneg ready
            scalar.wait_ge(ns_sem, 16)
            for c in range(K_S):
                if c >= 2:
                    scalar.wait_ge(t_sem, pos_of_s[c - 2] + 1)
                scalar.activation(
                    ms[c % 2][:, :],
                    tneg[:, :],
                    mybir.ActivationFunctionType.Relu,
                    bias=negs_sb[:, c : c + 1],
                    scale=1.0,
                ).then_inc(s_sem, 1)

        @block.vector
        def _(vector):
            vv = 0
            vector.wait_ge(p_sem, 16)
            vector.memset(ones_t[:, :], 1.0).then_inc(v_sem, 1)
            vv += 1
            # tneg = delta - pos   (fp16 out)
            vector.tensor_scalar(
                out=tneg[:, :],
                in0=pos_rep[:, :],
                scalar1=-1.0,
                scalar2=DELTA,
                op0=mybir.AluOpType.mult,
                op1=mybir.AluOpType.add,
            ).then_inc(v_sem, 1)
            vv += 1
            vector.wait_ge(nv_sem, 16)
            for k in range(K_V):
                if k >= 2:
                    vector.wait_ge(t_sem, pos_of_v[k - 2] + 1)
                vector.wait_ge(v_sem, vv)  # same-engine chain
                vector.tensor_scalar(
                    out=mv[k % 2][:, :],
                    in0=tneg[:, :],
                    scalar1=negv_sb[:, k : k + 1],
                    scalar2=0.0,
                    op0=mybir.AluOpType.add,
                    op1=mybir.AluOpType.max,
                ).then_inc(v_sem, 1)
                vv += 1
            # tail
            vector.wait_ge(t_sem, N_COLS)
            vector.tensor_copy(q_sb[0:1, :], psum_q[0:1, :]).then_inc(v_sem, 1)
            vv += 1
            vector.wait_ge(misc_sem, 32)
            vector.wait_ge(v_sem, vv)
            vector.scalar_tensor_tensor(
                out=newlam_sb[0:1, :],
                in0=q_sb[0:1, :],
                scalar=mu_sb[0:1, 0:1],
                in1=lam_sb[0:1, :],
                op0=mybir.AluOpType.mult,
                op1=mybir.AluOpType.add,
            ).then_inc(done_sem, 1)

        @block.tensor
        def _(tensor):
            for i, (t, idx) in enumerate(sched):
                if t == "v":
                    tensor.wait_ge(v_sem, 3 + idx)  # 2 prelim ops then col idx
                    buf = mv[idx % 2]
                else:
                    tensor.wait_ge(s_sem, idx + 1)
                    buf = ms[idx % 2]
                tensor.matmul(
                    psum_q[0:1, :],
                    ones_t[:, 0:1],
                    buf[:, :],
                    start=(i == 0),
                    stop=(i == N_COLS - 1),
                ).then_inc(t_sem, 1)

    return nc


_NC_CACHE = None
LAST_RESULT = None


def _get_nc():
    global _NC_CACHE
    if _NC_CACHE is None:
        _NC_CACHE = build_kernel()
    return _NC_CACHE


def _shard_inputs(pos, neg, lam, mu_np, idx_is_arange):
    NEG_V = 128 * K_V
    negv = np.ascontiguousarray(neg[0:NEG_V].reshape(128, K_V))
    negs = np.ascontiguousarray(neg[NEG_V:].reshape(128, K_S))
    in_maps = []
    for c in range(N_CORES):
        sl = slice(c * P_SHARD, (c + 1) * P_SHARD)
        lam_sh = lam[sl] if idx_is_arange else np.zeros(P_SHARD, dtype=np.float32)
        in_maps.append(
            {
                "pos_row": pos[sl].reshape(1, P_SHARD).copy(),
                "negv": negv,
                "negs": negs,
                "lam_row": lam_sh.reshape(1, P_SHARD).copy(),
                "mu": mu_np.copy(),
            }
        )
    return in_maps


def kernel(buffer_batch_pos, buffer_batch_neg, lambdas_index_buffer, lambdas, mu):
    pos = np.asarray(buffer_batch_pos, dtype=np.float32)
    neg = np.asarray(buffer_batch_neg, dtype=np.float32)
    idx = np.asarray(lambdas_index_buffer)
    lam = np.asarray(lambdas, dtype=np.float32)
    mu_np = np.asarray(mu, dtype=np.float32).reshape(1, 1)

    assert pos.shape == (P_TOTAL,) and neg.shape == (N_NEG,)

    idx_is_arange = bool(np.array_equal(idx, np.arange(P_TOTAL)))

    nc = _get_nc()
    in_maps = _shard_inputs(pos, neg, lam, mu_np, idx_is_arange)
    res = run_bass_kernel_spmd(nc, in_maps, core_ids=list(range(N_CORES)))
    global LAST_RESULT
    LAST_RESULT = res
    results = res.results

    q_full = np.concatenate(
        [results[c]["q_row"].reshape(P_SHARD) for c in range(N_CORES)]
    ).astype(np.float32)

    if idx_is_arange:
        new_lambdas = np.concatenate(
            [results[c]["newlam_row"].reshape(P_SHARD) for c in range(N_CORES)]
        ).astype(np.float32)
    else:
        new_lambdas = lam.copy()
        np.add.at(new_lambdas, idx, (mu_np.ravel()[0] * q_full).astype(np.float32))

    mu_s = np.float32(mu_np.ravel()[0])
    q_last = np.float32(q_full[-1])
    lam_last = np.float32(lam[idx[-1]])
    loss = np.float32(
        (mu_s / np.float32(2.0) * q_last**2 + lam_last * q_last)
        / np.float32(P_TOTAL * N_NEG)
    )
    return new_lambdas, np.asarray(loss, dtype=np.float32)


# revision 15
# speedup vs baseline: 1.2335x; 1.2335x over previous
"""Trainium2 Bass kernel for the ALM-terms loss function (v3).

Reference (P=4096 pos, N=8192 neg, delta=0.1):
    q[i]        = sum_j relu(neg[j] - pos[i] + delta)
    new_lambdas = lambdas.at[idx].add(mu * q)          # idx is arange
    loss        = (mu/2 * q[-1]^2 + lambdas[idx[-1]] * q[-1]) / (P*N)

Sharding: pos split 8-way (512 rows/core); neg replicated. No collectives.

Per-core algorithm (all orientation B, fp16 margins):
  - tneg[p, i] = delta - pos[i]  replicated across partitions ([128,512] fp16)
  - neg as 64 columns of 128 ([128, k]); VectorE computes
    m = max(tneg + neg_k, 0) (fp16, 4x mode) for K_V columns; ScalarE
    computes m = Relu(tneg + neg_c) via activation for the rest.
  - TensorE contracts each m column over partitions with a ones-vector
    (fp16 rhs, fp32 PSUM) accumulating all 64 columns into one
    PSUM row [1, 512] = q for this shard.
  - Tail: q -> SBUF, newlam = q*mu + lam in row layout, DMA out.
"""

import numpy as np

import concourse.bass as bass
import concourse.mybir as mybir
from concourse.bass_utils import run_bass_kernel_spmd

P_TOTAL = 4096
N_NEG = 8192
N_CORES = 8
P_SHARD = P_TOTAL // N_CORES       # 512
DELTA = 0.1

N_COLS = N_NEG // 128              # 64 columns of 128 negs
K_V = 43                           # vector columns
K_S = N_COLS - K_V                 # scalar columns

F32 = mybir.dt.float32
F16 = mybir.dt.float16
BF16 = mybir.dt.bfloat16


def _interleave_schedule():
    """TensorE consumption order: list of ('v'|'s', col_idx), Bresenham mix."""
    sched = []
    iv = isch = 0
    for _ in range(N_COLS):
        if isch * K_V >= iv * K_S - 1e-9 and iv < K_V:
            sched.append(("v", iv))
            iv += 1
        elif isch < K_S:
            sched.append(("s", isch))
            isch += 1
        else:
            sched.append(("v", iv))
            iv += 1
    return sched


def build_kernel():
    nc = bass.Bass()

    row_ext = nc.declare_dram_parameter("rowblob", [1, 2 * P_SHARD + 1], F32, False)
    neg_ext = nc.declare_dram_parameter("negblob", [128, N_COLS], F32, False)
    q_ext = nc.declare_dram_parameter("q_row", [1, P_SHARD], F32, True)
    newlam_ext = nc.declare_dram_parameter("newlam_row", [1, P_SHARD], F32, True)

    sched = _interleave_schedule()
    pos_of_v = {k: i for i, (t, k) in enumerate(sched) if t == "v"}
    pos_of_s = {c: i for i, (t, c) in enumerate(sched) if t == "s"}

    from contextlib import ExitStack

    with ExitStack() as ctx:
        sb = lambda name, shape, dt=F32: ctx.enter_context(
            nc.sbuf_tensor(name, shape, dt)
        )
        pos_rep = sb("pos_rep", [128, P_SHARD])
        tneg = sb("tneg", [128, P_SHARD], F16)
        neg_sb = sb("neg_sb", [128, N_COLS])
        mv = [sb("mv0", [128, P_SHARD], BF16), sb("mv1", [128, P_SHARD], BF16), sb("mv2", [128, P_SHARD], BF16)]
        ms = [sb("ms0", [128, P_SHARD], BF16), sb("ms1", [128, P_SHARD], BF16)]
        ones_t = sb("ones_t", [128, 1], BF16)
        q_sb = sb("q_sb", [1, P_SHARD])
        lammu_sb = sb("lammu_sb", [1, P_SHARD + 1])
        newlam_sb = sb("newlam_sb", [1, P_SHARD])
        dum_sb = sb("dum_sb", [1, 1])
        psum_q = ctx.enter_context(nc.psum_tensor("psum_q", [1, P_SHARD], F32))
        sem = lambda name: ctx.enter_context(nc.semaphore(name))
        p_sem = sem("p_sem")
        nv_sem = sem("nv_sem")
        ns_sem = sem("ns_sem")
        misc_sem = sem("misc_sem")
        v_sem = sem("v_sem")
        s_sem = sem("s_sem")
        t_sem = sem("t_sem")
        done_sem = sem("done_sem")
        out_sem = sem("out_sem")
        block = ctx.enter_context(nc.Block())

        @block.sync
        def _(sync):
            sync.dma_start(
                out=pos_rep[:, :],
                in_=row_ext[0:1, 0:P_SHARD].broadcast_to((128, P_SHARD)),
            ).then_inc(p_sem, 16)
            sync.dma_start(out=neg_sb[:, :], in_=neg_ext[:, :]).then_inc(nv_sem, 16)
            sync.dma_start(
                out=lammu_sb[0:1, :], in_=row_ext[0:1, P_SHARD:]
            ).then_inc(misc_sem, 16)
            sync.wait_ge(done_sem, 1)
            sync.dma_start(out=q_ext[0:1, :], in_=q_sb[0:1, :]).then_inc(out_sem, 16)
            sync.dma_start(out=newlam_ext[0:1, :], in_=newlam_sb[0:1, :]).then_inc(
                out_sem, 16
            )

        @block.scalar
        def _(scalar):
            # dummy ACT pulls the Relu table load off the critical path
            scalar.wait_ge(p_sem, 16)
            scalar.activation(
                dum_sb[0:1, 0:1],
                pos_rep[0:1, 0:1],
                mybir.ActivationFunctionType.Relu,
                bias=0.0,
                scale=1.0,
            )
            scalar.wait_ge(v_sem, 2)  # tneg ready
            scalar.wait_ge(nv_sem, 16)
            for c in range(K_S):
                if c >= 2:
                    scalar.wait_ge(t_sem, pos_of_s[c - 2] + 1)
                scalar.activation(
                    ms[c % 2][:, :],
                    tneg[:, :],
                    mybir.ActivationFunctionType.Relu,
                    bias=neg_sb[:, K_V + c : K_V + c + 1],
                    scale=1.0,
                ).then_inc(s_sem, 1)


        @block.vector
        def _(vector):
            vv = 0
            vector.wait_ge(p_sem, 16)
            vector.memset(ones_t[:, :], 1.0).then_inc(v_sem, 1)
            vv += 1
            # tneg = delta - pos   (fp16 out)
            vector.tensor_scalar(
                out=tneg[:, :],
                in0=pos_rep[:, :],
                scalar1=-1.0,
                scalar2=DELTA,
                op0=mybir.AluOpType.mult,
                op1=mybir.AluOpType.add,
            ).then_inc(v_sem, 1)
            vv += 1
            vector.wait_ge(nv_sem, 16)
            for k in range(K_V):
                if k >= 3:
                    vector.wait_ge(t_sem, pos_of_v[k - 3] + 1)
                vector.wait_ge(v_sem, vv)  # same-engine chain
                vector.tensor_scalar(
                    out=mv[k % 3][:, :],
                    in0=tneg[:, :],
                    scalar1=neg_sb[:, k : k + 1],
                    scalar2=0.0,
                    op0=mybir.AluOpType.add,
                    op1=mybir.AluOpType.max,
                ).then_inc(v_sem, 1)
                vv += 1
            # tail
            vector.wait_ge(t_sem, N_COLS)
            vector.tensor_copy(q_sb[0:1, :], psum_q[0:1, :]).then_inc(v_sem, 1)
            vv += 1
            vector.wait_ge(misc_sem, 16)
            vector.wait_ge(v_sem, vv)
            vector.scalar_tensor_tensor(
                out=newlam_sb[0:1, :],
                in0=q_sb[0:1, :],
                scalar=lammu_sb[0:1, P_SHARD : P_SHARD + 1],
                in1=lammu_sb[0:1, 0:P_SHARD],
                op0=mybir.AluOpType.mult,
                op1=mybir.AluOpType.add,
            ).then_inc(done_sem, 1)

        @block.tensor
        def _(tensor):
            for i, (t, idx) in enumerate(sched):
                if t == "v":
                    tensor.wait_ge(v_sem, 3 + idx)  # 2 prelim ops then col idx
                    buf = mv[idx % 3]
                else:
                    tensor.wait_ge(s_sem, idx + 1)
                    buf = ms[idx % 2]
                tensor.matmul(
                    psum_q[0:1, :],
                    ones_t[:, 0:1],
                    buf[:, :],
                    start=(i == 0),
                    stop=(i == N_COLS - 1),
                ).then_inc(t_sem, 1)

    return nc


_NC_CACHE = None
LAST_RESULT = None


def _get_nc():
    global _NC_CACHE
    if _NC_CACHE is None:
        _NC_CACHE = build_kernel()
    return _NC_CACHE


def _shard_inputs(pos, neg, lam, mu_np, idx_is_arange):
    NEG_V = 128 * K_V
    negblob = np.concatenate(
        [neg[0:NEG_V].reshape(128, K_V), neg[NEG_V:].reshape(128, K_S)], axis=1
    )
    negblob = np.ascontiguousarray(negblob)
    in_maps = []
    for c in range(N_CORES):
        sl = slice(c * P_SHARD, (c + 1) * P_SHARD)
        lam_sh = lam[sl] if idx_is_arange else np.zeros(P_SHARD, dtype=np.float32)
        rowblob = np.concatenate(
            [pos[sl], lam_sh, mu_np.ravel()]
        ).reshape(1, 2 * P_SHARD + 1)
        in_maps.append(
            {
                "rowblob": np.ascontiguousarray(rowblob),
                "negblob": negblob,
            }
        )
    return in_maps


def kernel(buffer_batch_pos, buffer_batch_neg, lambdas_index_buffer, lambdas, mu):
    pos = np.asarray(buffer_batch_pos, dtype=np.float32)
    neg = np.asarray(buffer_batch_neg, dtype=np.float32)
    idx = np.asarray(lambdas_index_buffer)
    lam = np.asarray(lambdas, dtype=np.float32)
    mu_np = np.asarray(mu, dtype=np.float32).reshape(1, 1)

    assert pos.shape == (P_TOTAL,) and neg.shape == (N_NEG,)

    idx_is_arange = bool(np.array_equal(idx, np.arange(P_TOTAL)))

    nc = _get_nc()
    in_maps = _shard_inputs(pos, neg, lam, mu_np, idx_is_arange)
    res = run_bass_kernel_spmd(nc, in_maps, core_ids=list(range(N_CORES)))
    global LAST_RESULT
    LAST_RESULT = res
    results = res.results

    q_full = np.concatenate(
        [results[c]["q_row"].reshape(P_SHARD) for c in range(N_CORES)]
    ).astype(np.float32)

    if idx_is_arange:
        new_lambdas = np.concatenate(
            [results[c]["newlam_row"].reshape(P_SHARD) for c in range(N_CORES)]
        ).astype(np.float32)
    else:
        new_lambdas = lam.copy()
        np.add.at(new_lambdas, idx, (mu_np.ravel()[0] * q_full).astype(np.float32))

    mu_s = np.float32(mu_np.ravel()[0])
    q_last = np.float32(q_full[-1])
    lam_last = np.float32(lam[idx[-1]])
    loss = np.float32(
        (mu_s / np.float32(2.0) * q_last**2 + lam_last * q_last)
        / np.float32(P_TOTAL * N_NEG)
    )
    return new_lambdas, np.asarray(loss, dtype=np.float32)


# revision 16
# speedup vs baseline: 1.3220x; 1.0718x over previous
"""Trainium2 Bass kernel for the ALM-terms loss function (final v11).

Hybrid orientation, tuned DMA/start-up:
  - VectorE (orientation B): K_V columns of 128 negs; m = max(tneg + neg_k, 0)
    (fp16, [128, 512]); TensorE ones-reduce accumulates into PSUM row
    [1, 512] = q_v.
  - ScalarE (orientation A): SIG negs, pre-cast to fp16 on host, broadcast
    to 128 partitions via two parallel DMA queues (gpsimd chunk0 small to
    start early, sync chunk1); activation(Relu, bias=delta-pos_p exact f32,
    accum_out) -> ss_parts [128, 8].
  - Outputs: q_row [1, 512] + ss_parts [128, 8]; host combines.
"""

import numpy as np

import concourse.bass as bass
import concourse.mybir as mybir
from concourse.bass_utils import run_bass_kernel_spmd


def _install_profile_hook_shim():
    """antenv.axon_hooks is absent in this container; provide it so
    run_bass_kernel_spmd(trace=True) / BASS_TRACE=1 can capture NTFF
    profiles instead of crashing.  Harmless when tracing is off."""
    import sys as _sys
    import types as _types

    if "antenv.axon_hooks" in _sys.modules:
        return
    try:
        import antenv as _antenv
        from trn_agent_boot.trn_boot import _ntff_profile_via_ctypes

        hook = _ntff_profile_via_ctypes("/opt/axon/libaxon_pjrt.so")
        mod = _types.ModuleType("antenv.axon_hooks")
        _store = {"h": hook}
        mod.set_axon_ntff_profile_hook = lambda h: _store.__setitem__("h", h)
        mod.get_axon_ntff_profile_hook = lambda: _store["h"]
        _sys.modules["antenv.axon_hooks"] = mod
        _antenv.axon_hooks = mod
    except Exception:
        pass


_install_profile_hook_shim()

P_TOTAL = 4096
N_NEG = 8192
N_CORES = 8
P_SHARD = P_TOTAL // N_CORES       # 512
N_GROUPS = 4
DELTA = 0.1

K_V = 36                           # vector columns of 128 negs
NEG_V = 128 * K_V                  # 3968
SIG = N_NEG - NEG_V                # 4224 negs on the scalar path
CH0 = 1280                         # first (early) chunk
CH1 = SIG - CH0                    # 2944

F32 = mybir.dt.float32
F16 = mybir.dt.float16

NEGB_W = K_V + N_GROUPS            # negblob: [negv | pos_cm]


def build_kernel():
    nc = bass.Bass()

    pos_ext = nc.declare_dram_parameter("pos_rep_in", [128, P_SHARD], F32, False)
    negs0_ext = nc.declare_dram_parameter("negs0", [128, CH0], F16, False)
    negs1_ext = nc.declare_dram_parameter("negs1", [128, CH1], F16, False)
    neg_ext = nc.declare_dram_parameter("negblob", [128, NEGB_W], F32, False)
    q_ext = nc.declare_dram_parameter("q_row", [1, P_SHARD], F32, True)
    ss_ext = nc.declare_dram_parameter("ss_parts", [128, 2 * N_GROUPS], F32, True)

    from contextlib import ExitStack

    with ExitStack() as ctx:
        sb = lambda name, shape, dt=F32: ctx.enter_context(
            nc.sbuf_tensor(name, shape, dt)
        )
        pos_rep = sb("pos_rep", [128, P_SHARD])
        tneg = sb("tneg", [128, P_SHARD], F16)
        neg_sb = sb("neg_sb", [128, NEGB_W])
        nt4 = sb("nt4", [128, N_GROUPS])
        negs_rep = sb("negs_rep", [128, SIG], F16)
        m_ring = sb("m_ring", [128, 3 * P_SHARD], F16)
        m_s = [sb("ms0", [128, CH1], F16), sb("ms1", [128, CH1], F16)]
        ss_sb = sb("ss_sb", [128, 2 * N_GROUPS])
        ones_t = sb("ones_t", [128, 1], F16)
        q_sb = sb("q_sb", [1, P_SHARD])
        dum_sb = sb("dum_sb", [1, 1])
        psum_q = ctx.enter_context(nc.psum_tensor("psum_q", [1, P_SHARD], F32))
        sem = lambda name: ctx.enter_context(nc.semaphore(name))
        nb_sem = sem("nb_sem")
        pr_sem = sem("pr_sem")
        b0_sem = sem("b0_sem")
        b1_sem = sem("b1_sem")
        v_sem = sem("v_sem")
        s_sem = sem("s_sem")
        t_sem = sem("t_sem")
        vdone_sem = sem("vdone_sem")
        out_sem = sem("out_sem")
        block = ctx.enter_context(nc.Block())

        @block.sync
        def _(sync):
            sync.dma_start(out=neg_sb[:, :], in_=neg_ext[:, :]).then_inc(nb_sem, 16)
            sync.dma_start(out=pos_rep[:, :], in_=pos_ext[:, :]).then_inc(pr_sem, 16)
            sync.dma_start(out=negs_rep[:, CH0:], in_=negs1_ext[:, :]).then_inc(
                b1_sem, 16
            )
            sync.wait_ge(vdone_sem, 1)
            sync.dma_start(out=q_ext[0:1, :], in_=q_sb[0:1, :]).then_inc(out_sem, 16)
            sync.wait_ge(s_sem, 2 * N_GROUPS)
            sync.dma_start(out=ss_ext[:, :], in_=ss_sb[:, :]).then_inc(out_sem, 16)

        @block.gpsimd
        def _(gpsimd):
            gpsimd.dma_start(out=negs_rep[:, 0:CH0], in_=negs0_ext[:, :]).then_inc(
                b0_sem, 16
            )

        @block.scalar
        def _(scalar):
            # dummy ACT pulls the Relu table load off the critical path
            scalar.wait_ge(nb_sem, 16)
            scalar.activation(
                dum_sb[0:1, 0:1],
                neg_sb[0:1, 0:1],
                mybir.ActivationFunctionType.Relu,
                bias=0.0,
                scale=1.0,
            )
            scalar.wait_ge(v_sem, 1)  # nt4 ready
            sv = 0
            for c, (lo, w) in enumerate([(0, CH0), (CH0, CH1)]):
                scalar.wait_ge([b0_sem, b1_sem][c], 16)
                for g in range(N_GROUPS):
                    if sv >= 2:
                        scalar.wait_ge(s_sem, sv - 1)  # WAW chain on m_s pair
                    scalar.activation(
                        m_s[sv % 2][:, 0:w],
                        negs_rep[:, lo : lo + w],
                        mybir.ActivationFunctionType.Relu,
                        bias=nt4[:, g : g + 1],
                        scale=1.0,
                        accum_out=ss_sb[:, c * N_GROUPS + g : c * N_GROUPS + g + 1],
                    ).then_inc(s_sem, 1)
                    sv += 1

        @block.vector
        def _(vector):
            vector.wait_ge(nb_sem, 16)
            # nt4 = delta - pos_cm (f32, bias for scalar path)
            vector.tensor_scalar(
                out=nt4[:, :],
                in0=neg_sb[:, K_V : K_V + N_GROUPS],
                scalar1=-1.0,
                scalar2=DELTA,
                op0=mybir.AluOpType.mult,
                op1=mybir.AluOpType.add,
            ).then_inc(v_sem, 1)
            vector.memset(ones_t[:, :], 1.0).then_inc(v_sem, 1)
            vector.wait_ge(pr_sem, 16)
            # tneg = delta - pos (fp16)
            vector.tensor_scalar(
                out=tneg[:, :],
                in0=pos_rep[:, :],
                scalar1=-1.0,
                scalar2=DELTA,
                op0=mybir.AluOpType.mult,
                op1=mybir.AluOpType.add,
            ).then_inc(v_sem, 1)
            for k in range(K_V):
                if k >= 3:
                    # t_sem wait also orders the tneg RAW transitively
                    vector.wait_ge(t_sem, k - 2)
                else:
                    vector.wait_ge(v_sem, 3)
                s = (k % 3) * P_SHARD
                vector.tensor_scalar(
                    out=m_ring[:, s : s + P_SHARD],
                    in0=tneg[:, :],
                    scalar1=neg_sb[:, k : k + 1],
                    scalar2=0.0,
                    op0=mybir.AluOpType.add,
                    op1=mybir.AluOpType.max,
                ).then_inc(v_sem, 1)
            # tail: q_v out of PSUM
            vector.wait_ge(t_sem, K_V)
            vector.tensor_copy(q_sb[0:1, :], psum_q[0:1, :]).then_inc(vdone_sem, 1)

        @block.tensor
        def _(tensor):
            for k in range(K_V):
                tensor.wait_ge(v_sem, 4 + k)
                s = (k % 3) * P_SHARD
                tensor.matmul(
                    psum_q[0:1, :],
                    ones_t[:, 0:1],
                    m_ring[:, s : s + P_SHARD],
                    start=(k == 0),
                    stop=(k == K_V - 1),
                ).then_inc(t_sem, 1)

    return nc


_NC_CACHE = None
LAST_RESULT = None


def _get_nc():
    global _NC_CACHE
    if _NC_CACHE is None:
        _NC_CACHE = build_kernel()
    return _NC_CACHE


def _shard_inputs(pos, neg, lam, mu_np, idx_is_arange):
    negv = neg[0:NEG_V].reshape(128, K_V)
    negs16 = neg[NEG_V:].astype(np.float16)
    negs0 = np.ascontiguousarray(np.broadcast_to(negs16[0:CH0], (128, CH0)))
    negs1 = np.ascontiguousarray(np.broadcast_to(negs16[CH0:], (128, CH1)))
    in_maps = []
    for c in range(N_CORES):
        sl = slice(c * P_SHARD, (c + 1) * P_SHARD)
        pos_sh = pos[sl]
        pos_cm = pos_sh.reshape(N_GROUPS, 128).T
        negblob = np.ascontiguousarray(
            np.concatenate([negv, pos_cm], axis=1), dtype=np.float32
        )
        pos_rep = np.ascontiguousarray(
            np.broadcast_to(pos_sh.reshape(1, P_SHARD), (128, P_SHARD))
        )
        in_maps.append(
            {
                "pos_rep_in": pos_rep,
                "negs0": negs0,
                "negs1": negs1,
                "negblob": negblob,
            }
        )
    return in_maps


def kernel(buffer_batch_pos, buffer_batch_neg, lambdas_index_buffer, lambdas, mu):
    pos = np.asarray(buffer_batch_pos, dtype=np.float32)
    neg = np.asarray(buffer_batch_neg, dtype=np.float32)
    idx = np.asarray(lambdas_index_buffer)
    lam = np.asarray(lambdas, dtype=np.float32)
    mu_np = np.asarray(mu, dtype=np.float32).reshape(1, 1)

    assert pos.shape == (P_TOTAL,) and neg.shape == (N_NEG,)
    idx_is_arange = bool(np.array_equal(idx, np.arange(P_TOTAL)))

    nc = _get_nc()
    in_maps = _shard_inputs(pos, neg, lam, mu_np, idx_is_arange)
    res = run_bass_kernel_spmd(nc, in_maps, core_ids=list(range(N_CORES)))
    global LAST_RESULT
    LAST_RESULT = res
    results = res.results

    mu_s = np.float32(mu_np.ravel()[0])
    q_parts = []
    for c in range(N_CORES):
        qv = results[c]["q_row"].reshape(P_SHARD).astype(np.float32)
        ss = results[c]["ss_parts"].astype(np.float32)
        ss_tot = ss[:, 0:N_GROUPS] + ss[:, N_GROUPS:]
        q_parts.append(qv + ss_tot.T.reshape(P_SHARD))
    q_full = np.concatenate(q_parts).astype(np.float32)

    if idx_is_arange:
        new_lambdas = (lam + mu_s * q_full).astype(np.float32)
    else:
        new_lambdas = lam.copy()
        np.add.at(new_lambdas, idx, (mu_s * q_full).astype(np.float32))

    q_last = np.float32(q_full[-1])
    lam_last = np.float32(lam[idx[-1]])
    loss = np.float32(
        (mu_s / np.float32(2.0) * q_last**2 + lam_last * q_last)
        / np.float32(P_TOTAL * N_NEG)
    )
    return new_lambdas, np.asarray(loss, dtype=np.float32)


# revision 17
# speedup vs baseline: 1.3678x; 1.0346x over previous
"""Trainium2 Bass kernel for the ALM-terms loss function (final v20).

Hybrid orientation, tuned DMA/start-up:
  - VectorE (orientation B): K_V columns of 128 negs; m = max(tneg + neg_k, 0)
    (fp16, [128, 512]); TensorE ones-reduce accumulates into PSUM row
    [1, 512] = q_v.
  - ScalarE (orientation A): SIG negs, pre-cast to fp16 on host, broadcast
    to 128 partitions via two parallel DMA queues (gpsimd chunk0 small to
    start early, sync chunk1); activation(Relu, bias=delta-pos_p exact f32,
    accum_out) -> ss_parts [128, 8].
  - Outputs: q_row [1, 512] + ss_parts [128, 8]; host combines.
"""

import numpy as np

import concourse.bass as bass
import concourse.mybir as mybir
from concourse.bass_utils import run_bass_kernel_spmd


def _install_profile_hook_shim():
    """antenv.axon_hooks is absent in this container; provide it so
    run_bass_kernel_spmd(trace=True) / BASS_TRACE=1 can capture NTFF
    profiles instead of crashing.  Harmless when tracing is off."""
    import sys as _sys
    import types as _types

    if "antenv.axon_hooks" in _sys.modules:
        return
    try:
        import antenv as _antenv
        from trn_agent_boot.trn_boot import _ntff_profile_via_ctypes

        hook = _ntff_profile_via_ctypes("/opt/axon/libaxon_pjrt.so")
        mod = _types.ModuleType("antenv.axon_hooks")
        _store = {"h": hook}
        mod.set_axon_ntff_profile_hook = lambda h: _store.__setitem__("h", h)
        mod.get_axon_ntff_profile_hook = lambda: _store["h"]
        _sys.modules["antenv.axon_hooks"] = mod
        _antenv.axon_hooks = mod
    except Exception:
        pass


_install_profile_hook_shim()

P_TOTAL = 4096
N_NEG = 8192
N_CORES = 8
P_SHARD = P_TOTAL // N_CORES       # 512
N_GROUPS = 4
DELTA = 0.1

K_V = 40                           # vector columns of 128 negs
NEG_V = 128 * K_V                  # 3968
SIG = N_NEG - NEG_V                # 4224 negs on the scalar path
CH0 = 1280                         # first (early) chunk
CH1 = SIG - CH0                    # 2944

F32 = mybir.dt.float32
F16 = mybir.dt.float16

NEGB_W = K_V + N_GROUPS            # negblob: [negv | pos_cm]


def build_kernel():
    nc = bass.Bass()

    pos_ext = nc.declare_dram_parameter("pos_rep_in", [128, P_SHARD], F16, False)
    negs0_ext = nc.declare_dram_parameter("negs0", [128, CH0], F16, False)
    negs1_ext = nc.declare_dram_parameter("negs1", [128, CH1], F16, False)
    neg_ext = nc.declare_dram_parameter("negblob", [128, NEGB_W], F32, False)
    q_ext = nc.declare_dram_parameter("q_row", [1, P_SHARD], F32, True)
    ss_ext = nc.declare_dram_parameter("ss_parts", [128, 2 * N_GROUPS], F32, True)

    from contextlib import ExitStack

    with ExitStack() as ctx:
        sb = lambda name, shape, dt=F32: ctx.enter_context(
            nc.sbuf_tensor(name, shape, dt)
        )
        pos_rep = sb("pos_rep", [128, P_SHARD], F16)
        tneg = sb("tneg", [128, P_SHARD], F16)
        neg_sb = sb("neg_sb", [128, NEGB_W])
        nt4 = sb("nt4", [128, N_GROUPS])
        negs_rep = sb("negs_rep", [128, SIG], F16)
        m_ring = sb("m_ring", [128, 12 * P_SHARD], F16)
        dum512 = sb("dum512", [128, P_SHARD], F16)
        m_s = [sb("ms0", [128, CH1], F16), sb("ms1", [128, CH1], F16)]
        ss_sb = sb("ss_sb", [128, 2 * N_GROUPS])
        ones_t = sb("ones_t", [128, 1], F16)
        q_sb = sb("q_sb", [1, P_SHARD])
        dum_sb = sb("dum_sb", [1, 1])
        psum_q = ctx.enter_context(nc.psum_tensor("psum_q", [1, P_SHARD], F32))
        psum_w = ctx.enter_context(nc.psum_tensor("psum_w", [1, P_SHARD], F32))
        sem = lambda name: ctx.enter_context(nc.semaphore(name))
        nb_sem = sem("nb_sem")
        pr_sem = sem("pr_sem")
        b0_sem = sem("b0_sem")
        b1_sem = sem("b1_sem")
        v_sem = sem("v_sem")
        s_sem = sem("s_sem")
        t_sem = sem("t_sem")
        vdone_sem = sem("vdone_sem")
        out_sem = sem("out_sem")
        block = ctx.enter_context(nc.Block())

        @block.sync
        def _(sync):
            sync.dma_start(out=neg_sb[:, :], in_=neg_ext[:, :]).then_inc(nb_sem, 16)
            sync.dma_start(out=pos_rep[:, :], in_=pos_ext[:, :]).then_inc(pr_sem, 16)
            sync.dma_start(out=negs_rep[:, CH0:], in_=negs1_ext[:, :]).then_inc(
                b1_sem, 16
            )
            sync.wait_ge(vdone_sem, 1)
            sync.dma_start(out=q_ext[0:1, :], in_=q_sb[0:1, :]).then_inc(out_sem, 16)
            sync.wait_ge(s_sem, 2 * N_GROUPS)
            sync.dma_start(out=ss_ext[:, :], in_=ss_sb[:, :]).then_inc(out_sem, 16)

        @block.gpsimd
        def _(gpsimd):
            gpsimd.dma_start(out=negs_rep[:, 0:CH0], in_=negs0_ext[:, :]).then_inc(
                b0_sem, 16
            )

        @block.scalar
        def _(scalar):
            # dummy ACT pulls the Relu table load off the critical path
            scalar.wait_ge(v_sem, 1)
            scalar.activation(
                dum_sb[0:1, 0:1],
                ones_t[0:1, 0:1],
                mybir.ActivationFunctionType.Relu,
                bias=0.0,
                scale=1.0,
            )
            scalar.wait_ge(v_sem, 3)  # nt4 ready
            sv = 0
            for c, (lo, w) in enumerate([(0, CH0), (CH0, CH1)]):
                scalar.wait_ge([b0_sem, b1_sem][c], 16)
                for g in range(N_GROUPS):
                    if sv >= 2:
                        scalar.wait_ge(s_sem, sv - 1)  # WAW chain on m_s pair
                    scalar.activation(
                        m_s[sv % 2][:, 0:w],
                        negs_rep[:, lo : lo + w],
                        mybir.ActivationFunctionType.Relu,
                        bias=nt4[:, g : g + 1],
                        scale=1.0,
                        accum_out=ss_sb[:, c * N_GROUPS + g : c * N_GROUPS + g + 1],
                    ).then_inc(s_sem, 1)
                    sv += 1

        @block.vector
        def _(vector):
            vector.memset(ones_t[:, :], 1.0).then_inc(v_sem, 1)
            vector.memset(dum512[:, :], 0.5).then_inc(v_sem, 1)
            vector.wait_ge(nb_sem, 16)
            # nt4 = delta - pos_cm (f32, bias for scalar path)
            vector.tensor_scalar(
                out=nt4[:, :],
                in0=neg_sb[:, K_V : K_V + N_GROUPS],
                scalar1=-1.0,
                scalar2=DELTA,
                op0=mybir.AluOpType.mult,
                op1=mybir.AluOpType.add,
            ).then_inc(v_sem, 1)
            vector.wait_ge(pr_sem, 16)
            # tneg = delta - pos (fp16)
            vector.tensor_scalar(
                out=tneg[:, :],
                in0=pos_rep[:, :],
                scalar1=-1.0,
                scalar2=DELTA,
                op0=mybir.AluOpType.mult,
                op1=mybir.AluOpType.add,
            ).then_inc(v_sem, 1)
            for k in range(K_V):
                if k >= 6:
                    # t_sem wait also orders the tneg RAW transitively
                    vector.wait_ge(t_sem, 8 + k - 5)
                else:
                    vector.wait_ge(v_sem, 4)
                s = (2 * (k % 6) + 1) * P_SHARD
                vector.tensor_scalar(
                    out=m_ring[:, s : s + P_SHARD],
                    in0=tneg[:, :],
                    scalar1=neg_sb[:, k : k + 1],
                    scalar2=0.0,
                    op0=mybir.AluOpType.add,
                    op1=mybir.AluOpType.max,
                ).then_inc(v_sem, 1)
            # tail: q_v out of PSUM
            vector.wait_ge(t_sem, 8 + K_V)
            vector.tensor_copy(q_sb[0:1, :], psum_q[0:1, :]).then_inc(vdone_sem, 1)

        @block.tensor
        def _(tensor):
            # HAM warm-up: keep the PE busy during the input-DMA window so the
            # clock gate opens before the real reduction starts
            tensor.wait_ge(v_sem, 2)
            for w in range(8):
                tensor.matmul(
                    psum_w[0:1, :],
                    ones_t[:, 0:1],
                    dum512[:, :],
                    start=True,
                    stop=True,
                ).then_inc(t_sem, 1)
            for k in range(K_V):
                tensor.wait_ge(v_sem, 5 + k)
                s = (2 * (k % 6) + 1) * P_SHARD
                tensor.matmul(
                    psum_q[0:1, :],
                    ones_t[:, 0:1],
                    m_ring[:, s : s + P_SHARD],
                    start=(k == 0),
                    stop=(k == K_V - 1),
                ).then_inc(t_sem, 1)

    return nc


_NC_CACHE = None
LAST_RESULT = None


def _get_nc():
    global _NC_CACHE
    if _NC_CACHE is None:
        _NC_CACHE = build_kernel()
    return _NC_CACHE


def _shard_inputs(pos, neg, lam, mu_np, idx_is_arange):
    negv = neg[0:NEG_V].reshape(128, K_V)
    negs16 = neg[NEG_V:].astype(np.float16)
    negs0 = np.ascontiguousarray(np.broadcast_to(negs16[0:CH0], (128, CH0)))
    negs1 = np.ascontiguousarray(np.broadcast_to(negs16[CH0:], (128, CH1)))
    in_maps = []
    for c in range(N_CORES):
        sl = slice(c * P_SHARD, (c + 1) * P_SHARD)
        pos_sh = pos[sl]
        pos_cm = pos_sh.reshape(N_GROUPS, 128).T
        negblob = np.ascontiguousarray(
            np.concatenate([negv, pos_cm], axis=1), dtype=np.float32
        )
        pos_rep = np.ascontiguousarray(
            np.broadcast_to(
                pos_sh.astype(np.float16).reshape(1, P_SHARD), (128, P_SHARD)
            )
        )
        in_maps.append(
            {
                "pos_rep_in": pos_rep,
                "negs0": negs0,
                "negs1": negs1,
                "negblob": negblob,
            }
        )
    return in_maps


def kernel(buffer_batch_pos, buffer_batch_neg, lambdas_index_buffer, lambdas, mu):
    pos = np.asarray(buffer_batch_pos, dtype=np.float32)
    neg = np.asarray(buffer_batch_neg, dtype=np.float32)
    idx = np.asarray(lambdas_index_buffer)
    lam = np.asarray(lambdas, dtype=np.float32)
    mu_np = np.asarray(mu, dtype=np.float32).reshape(1, 1)

    assert pos.shape == (P_TOTAL,) and neg.shape == (N_NEG,)
    idx_is_arange = bool(np.array_equal(idx, np.arange(P_TOTAL)))

    nc = _get_nc()
    in_maps = _shard_inputs(pos, neg, lam, mu_np, idx_is_arange)
    res = run_bass_kernel_spmd(nc, in_maps, core_ids=list(range(N_CORES)))
    global LAST_RESULT
    LAST_RESULT = res
    results = res.results

    mu_s = np.float32(mu_np.ravel()[0])
    q_parts = []
    for c in range(N_CORES):
        qv = results[c]["q_row"].reshape(P_SHARD).astype(np.float32)
        ss = results[c]["ss_parts"].astype(np.float32)
        ss_tot = ss[:, 0:N_GROUPS] + ss[:, N_GROUPS:]
        q_parts.append(qv + ss_tot.T.reshape(P_SHARD))
    q_full = np.concatenate(q_parts).astype(np.float32)

    if idx_is_arange:
        new_lambdas = (lam + mu_s * q_full).astype(np.float32)
    else:
        new_lambdas = lam.copy()
        np.add.at(new_lambdas, idx, (mu_s * q_full).astype(np.float32))

    q_last = np.float32(q_full[-1])
    lam_last = np.float32(lam[idx[-1]])
    loss = np.float32(
        (mu_s / np.float32(2.0) * q_last**2 + lam_last * q_last)
        / np.float32(P_TOTAL * N_NEG)
    )
    return new_lambdas, np.asarray(loss, dtype=np.float32)


# revision 18
# speedup vs baseline: 1.6263x; 1.1890x over previous
"""Trainium2 Bass kernel for the ALM-terms loss function (final v21).

Hybrid orientation, tuned DMA/start-up:
  - VectorE (orientation B): K_V columns of 128 negs; m = max(tneg + neg_k, 0)
    (fp16, [128, 512]); TensorE ones-reduce accumulates into PSUM row
    [1, 512] = q_v.
  - ScalarE (orientation A): SIG negs, pre-cast to fp16 on host, broadcast
    to 128 partitions via two parallel DMA queues (gpsimd chunk0 small to
    start early, sync chunk1); activation(Relu, bias=delta-pos_p exact f32,
    accum_out) -> ss_parts [128, 8].
  - Outputs: q_row [1, 512] + ss_parts [128, 8]; host combines.
"""

import numpy as np

import concourse.bass as bass
import concourse.mybir as mybir
from concourse.bass_utils import run_bass_kernel_spmd


def _install_profile_hook_shim():
    """antenv.axon_hooks is absent in this container; provide it so
    run_bass_kernel_spmd(trace=True) / BASS_TRACE=1 can capture NTFF
    profiles instead of crashing.  Harmless when tracing is off."""
    import sys as _sys
    import types as _types

    if "antenv.axon_hooks" in _sys.modules:
        return
    try:
        import antenv as _antenv
        from trn_agent_boot.trn_boot import _ntff_profile_via_ctypes

        hook = _ntff_profile_via_ctypes("/opt/axon/libaxon_pjrt.so")
        mod = _types.ModuleType("antenv.axon_hooks")
        _store = {"h": hook}
        mod.set_axon_ntff_profile_hook = lambda h: _store.__setitem__("h", h)
        mod.get_axon_ntff_profile_hook = lambda: _store["h"]
        _sys.modules["antenv.axon_hooks"] = mod
        _antenv.axon_hooks = mod
    except Exception:
        pass


_install_profile_hook_shim()

P_TOTAL = 4096
N_NEG = 8192
N_CORES = 8
P_SHARD = P_TOTAL // N_CORES       # 512
N_GROUPS = 4
DELTA = 0.1

K_V = 42                           # vector columns of 128 negs
NEG_V = 128 * K_V                  # 3968
SIG = N_NEG - NEG_V                # 4224 negs on the scalar path


F32 = mybir.dt.float32
F16 = mybir.dt.float16

NEGB_W = K_V + N_GROUPS            # negblob: [negv | pos_cm]


def build_kernel():
    nc = bass.Bass()

    pos_ext = nc.declare_dram_parameter("pos_rep_in", [128, P_SHARD], F16, False)
    negs_ext = nc.declare_dram_parameter("negs", [128, SIG], F16, False)
    neg_ext = nc.declare_dram_parameter("negblob", [128, NEGB_W], F32, False)
    q_ext = nc.declare_dram_parameter("q_row", [1, P_SHARD], F32, True)
    ss_ext = nc.declare_dram_parameter("ss_parts", [128, N_GROUPS], F32, True)

    from contextlib import ExitStack

    with ExitStack() as ctx:
        sb = lambda name, shape, dt=F32: ctx.enter_context(
            nc.sbuf_tensor(name, shape, dt)
        )
        pos_rep = sb("pos_rep", [128, P_SHARD], F16)
        tneg = sb("tneg", [128, P_SHARD], F16)
        neg_sb = sb("neg_sb", [128, NEGB_W])
        nt4 = sb("nt4", [128, N_GROUPS])
        negs_rep = sb("negs_rep", [128, SIG], F16)
        m_ring = sb("m_ring", [128, 12 * P_SHARD], F16)
        dum512 = sb("dum512", [128, P_SHARD], F16)
        m_s = [sb("ms0", [128, SIG], F16), sb("ms1", [128, SIG], F16)]
        ss_sb = sb("ss_sb", [128, N_GROUPS])
        ones_t = sb("ones_t", [128, 1], F16)
        q_sb = sb("q_sb", [1, P_SHARD])
        dum_sb = sb("dum_sb", [1, 1])
        psum_q = ctx.enter_context(nc.psum_tensor("psum_q", [1, P_SHARD], F32))
        psum_w = ctx.enter_context(nc.psum_tensor("psum_w", [1, P_SHARD], F32))
        sem = lambda name: ctx.enter_context(nc.semaphore(name))
        nb_sem = sem("nb_sem")
        pr_sem = sem("pr_sem")
        b1_sem = sem("b1_sem")
        v_sem = sem("v_sem")
        s_sem = sem("s_sem")
        t_sem = sem("t_sem")
        vdone_sem = sem("vdone_sem")
        out_sem = sem("out_sem")
        block = ctx.enter_context(nc.Block())

        @block.sync
        def _(sync):
            sync.dma_start(out=neg_sb[:, :], in_=neg_ext[:, :]).then_inc(nb_sem, 16)
            sync.dma_start(out=pos_rep[:, :], in_=pos_ext[:, :]).then_inc(pr_sem, 16)
            sync.dma_start(out=negs_rep[:, :], in_=negs_ext[:, :]).then_inc(
                b1_sem, 16
            )
            sync.wait_ge(vdone_sem, 1)
            sync.dma_start(out=q_ext[0:1, :], in_=q_sb[0:1, :]).then_inc(out_sem, 16)
            sync.wait_ge(s_sem, N_GROUPS)
            sync.dma_start(out=ss_ext[:, :], in_=ss_sb[:, :]).then_inc(out_sem, 16)

        @block.scalar
        def _(scalar):
            # dummy ACT pulls the Relu table load off the critical path
            scalar.wait_ge(v_sem, 1)
            scalar.activation(
                dum_sb[0:1, 0:1],
                ones_t[0:1, 0:1],
                mybir.ActivationFunctionType.Relu,
                bias=0.0,
                scale=1.0,
            )
            scalar.wait_ge(v_sem, 3)  # nt4 ready
            scalar.wait_ge(b1_sem, 16)
            for g in range(N_GROUPS):
                if g >= 2:
                    scalar.wait_ge(s_sem, g - 1)  # WAW chain on m_s pair
                scalar.activation(
                    m_s[g % 2][:, :],
                    negs_rep[:, :],
                    mybir.ActivationFunctionType.Relu,
                    bias=nt4[:, g : g + 1],
                    scale=1.0,
                    accum_out=ss_sb[:, g : g + 1],
                ).then_inc(s_sem, 1)

        @block.vector
        def _(vector):
            vector.memset(ones_t[:, :], 1.0).then_inc(v_sem, 1)
            vector.memset(dum512[:, :], 0.5).then_inc(v_sem, 1)
            vector.wait_ge(nb_sem, 16)
            # nt4 = delta - pos_cm (f32, bias for scalar path)
            vector.tensor_scalar(
                out=nt4[:, :],
                in0=neg_sb[:, K_V : K_V + N_GROUPS],
                scalar1=-1.0,
                scalar2=DELTA,
                op0=mybir.AluOpType.mult,
                op1=mybir.AluOpType.add,
            ).then_inc(v_sem, 1)
            vector.wait_ge(pr_sem, 16)
            # tneg = delta - pos (fp16)
            vector.tensor_scalar(
                out=tneg[:, :],
                in0=pos_rep[:, :],
                scalar1=-1.0,
                scalar2=DELTA,
                op0=mybir.AluOpType.mult,
                op1=mybir.AluOpType.add,
            ).then_inc(v_sem, 1)
            for k in range(K_V):
                if k >= 6:
                    # t_sem wait also orders the tneg RAW transitively
                    vector.wait_ge(t_sem, 8 + k - 5)
                else:
                    vector.wait_ge(v_sem, 4)
                s = (2 * (k % 6) + 1) * P_SHARD
                vector.tensor_scalar(
                    out=m_ring[:, s : s + P_SHARD],
                    in0=tneg[:, :],
                    scalar1=neg_sb[:, k : k + 1],
                    scalar2=0.0,
                    op0=mybir.AluOpType.add,
                    op1=mybir.AluOpType.max,
                ).then_inc(v_sem, 1)
            # tail: q_v out of PSUM
            vector.wait_ge(t_sem, 8 + K_V)
            vector.tensor_copy(q_sb[0:1, :], psum_q[0:1, :]).then_inc(vdone_sem, 1)

        @block.tensor
        def _(tensor):
            # HAM warm-up: keep the PE busy during the input-DMA window so the
            # clock gate opens before the real reduction starts
            tensor.wait_ge(v_sem, 2)
            for w in range(8):
                tensor.matmul(
                    psum_w[0:1, :],
                    ones_t[:, 0:1],
                    dum512[:, :],
                    start=True,
                    stop=True,
                ).then_inc(t_sem, 1)
            for k in range(K_V):
                tensor.wait_ge(v_sem, 5 + k)
                s = (2 * (k % 6) + 1) * P_SHARD
                tensor.matmul(
                    psum_q[0:1, :],
                    ones_t[:, 0:1],
                    m_ring[:, s : s + P_SHARD],
                    start=(k == 0),
                    stop=(k == K_V - 1),
                ).then_inc(t_sem, 1)

    return nc


_NC_CACHE = None
LAST_RESULT = None


def _get_nc():
    global _NC_CACHE
    if _NC_CACHE is None:
        _NC_CACHE = build_kernel()
    return _NC_CACHE


def _shard_inputs(pos, neg, lam, mu_np, idx_is_arange):
    negv = neg[0:NEG_V].reshape(128, K_V)
    negs16 = neg[NEG_V:].astype(np.float16)
    negs_r = np.ascontiguousarray(np.broadcast_to(negs16, (128, SIG)))
    in_maps = []
    for c in range(N_CORES):
        sl = slice(c * P_SHARD, (c + 1) * P_SHARD)
        pos_sh = pos[sl]
        pos_cm = pos_sh.reshape(N_GROUPS, 128).T
        negblob = np.ascontiguousarray(
            np.concatenate([negv, pos_cm], axis=1), dtype=np.float32
        )
        pos_rep = np.ascontiguousarray(
            np.broadcast_to(
                pos_sh.astype(np.float16).reshape(1, P_SHARD), (128, P_SHARD)
            )
        )
        in_maps.append(
            {
                "pos_rep_in": pos_rep,
                "negs": negs_r,
                "negblob": negblob,
            }
        )
    return in_maps


def kernel(buffer_batch_pos, buffer_batch_neg, lambdas_index_buffer, lambdas, mu):
    pos = np.asarray(buffer_batch_pos, dtype=np.float32)
    neg = np.asarray(buffer_batch_neg, dtype=np.float32)
    idx = np.asarray(lambdas_index_buffer)
    lam = np.asarray(lambdas, dtype=np.float32)
    mu_np = np.asarray(mu, dtype=np.float32).reshape(1, 1)

    assert pos.shape == (P_TOTAL,) and neg.shape == (N_NEG,)
    idx_is_arange = bool(np.array_equal(idx, np.arange(P_TOTAL)))

    nc = _get_nc()
    in_maps = _shard_inputs(pos, neg, lam, mu_np, idx_is_arange)
    res = run_bass_kernel_spmd(nc, in_maps, core_ids=list(range(N_CORES)))
    global LAST_RESULT
    LAST_RESULT = res
    results = res.results

    mu_s = np.float32(mu_np.ravel()[0])
    q_parts = []
    for c in range(N_CORES):
        qv = results[c]["q_row"].reshape(P_SHARD).astype(np.float32)
        ss_tot = results[c]["ss_parts"].astype(np.float32)
        q_parts.append(qv + ss_tot.T.reshape(P_SHARD))
    q_full = np.concatenate(q_parts).astype(np.float32)

    if idx_is_arange:
        new_lambdas = (lam + mu_s * q_full).astype(np.float32)
    else:
        new_lambdas = lam.copy()
        np.add.at(new_lambdas, idx, (mu_s * q_full).astype(np.float32))

    q_last = np.float32(q_full[-1])
    lam_last = np.float32(lam[idx[-1]])
    loss = np.float32(
        (mu_s / np.float32(2.0) * q_last**2 + lam_last * q_last)
        / np.float32(P_TOTAL * N_NEG)
    )
    return new_lambdas, np.asarray(loss, dtype=np.float32)
